# revision 89
# baseline (speedup 1.0000x reference)
"""Trainium2 Bass kernel for nn_BAGDnet (gnn_message_passing).

Computation (per measurement m):
    T = tKF[meas_kf[m]]          # 4x4 pose
    p = tMP[meas_mp[m]]          # 3d map point
    pts = T[:3] @ [p, 1]
    out[m] = (pts0/pts2*FX + CX, pts1/pts2*FY + CY)

Strategy: fold the intrinsics into per-keyframe rows host-side
    A = FX*T[0] + CX*T[2],  B = FY*T[1] + CY*T[2],  C = T[2]
so out = (A.h/C.h, B.h/C.h) with h = [p, 1].  Measurements are sorted by
keyframe and packed into blocks of 32 keyframes; each block becomes one
block-diagonal fp16 weight matrix (contraction rows 3j..3j+2 hold kf j's
point coords, row 96 is a shared ones-row carrying the translation).

Per column chunk the Tensor engine computes (a,b,c) for 32 measurements.
Walrus allows only one PSUM operand per DVE op and the elementwise
engines charge by free-dim columns regardless of partitions, so the
kernel packs TWO <=512-column chunks into every 128-partition
elementwise pass (pair mode): 64-col-weight matmuls write chunk A's
ab-plane to PSUM partitions 0:64 and chunk B's to 64:128, their
duplicated-c planes to a second one-bank tile (divisor matmuls emitted
first so the eviction starts early); one ACT eviction and one fused DVE
divide (Newton-Raphson reciprocal * numerator) then retire both chunks,
halving the per-column ACT/DVE cost at the price of a second PE pass
(PE runs at full clock and has slack).  Eight one-bank PSUM tiles keep
four groups in flight.  Block 0 ramps with two small single-mode chunks
so the divide pipeline starts as soon as the first small load lands
(single-mode outputs start at partition 0 -- hardware silently breaks
custom-DVE writes at partition base 64).  Block 7 ends with a tiny
(64,64) group so the final dependency chain is short.  Loads are nine
block-aligned column-span DMAs from SP (fine splits keep the divide
engines fed); stores are six og-ranges: SP early, Pool for the
second-to-last-but-one, and the Activation queue (idle once evictions
finish) for the last tiny range, so the dependency waits all overlap.
"""

import numpy as np

N_CORES = 8
N_KF = 2000
KF_PER_BLK = 32            # keyframes per weight block (32*3+1 = 97 rows)
NBLK = 8                   # weight blocks per core  (8*8*32 = 2048 >= 2000)
NRANK = N_CORES * NBLK * KF_PER_BLK
CROWS = 97                 # contraction rows: 96 coord rows + ones row
WCOLS = 128                # weight free dim: 64 a/b cols + 64 c/c cols
PW = 512                   # max chunk width (one PSUM bank per 64-row plane)
# extra blocks processed in single mode (block 0 always is); balances the
# Tensor engine's second pair-mode pass against DVE divide throughput
SINGLE_BLOCKS = (2,)
FX = 320.0
FY = 320.0
CX = 320.0
CY = 240.0

_CACHE = {}


def _fused_div_op():
    """Register (once) a custom DVE op: out = Src1 * approx(1/Src0).

    BITWISE_NOT exponent-flip seed + one Newton-Raphson pass + multiply by
    Src1 in a single DVE instruction.  ~1.7e-3 max rel err for Src0 in
    [2, 8], the entire range of the projective divisor here."""
    import numpy as np

    import concourse.dve_ops as dops
    from concourse.dve_spec import AluOp, Bin, C0, C1, Spec, Src0, Src1, lower
    from concourse.dve_uop import DveOpSpec

    NAME = "DIV_APPROX_ANT"
    for op in dops.OPS:
        if op.name == NAME:
            return op

    _nx = Bin(AluOp.BITWISE_NOT, Src0, Src0)
    _y0 = _nx * C0
    _y1 = _y0 * (C1 - Src0 * _y0)

    def _ref(in0, in1, c0, c1, c2):
        nx = (~in0.view(np.int32)).view(np.float32)
        y0 = nx * c0
        y1 = y0 * (c1 - in0 * y0)
        return y1 * in1

    spec = Spec(body=_y1 * Src1, reference=_ref)
    row = dops._CUSTOM_DVE_ROW_BASE + len(dops.OPS)
    shas = {}
    for ver in ("v3", "v4"):
        shas[ver] = DveOpSpec(name=NAME, opcode=row, uops=lower(spec, ver=ver),
                              rd1_en=True).sha(ver)
    op = dops.DveOp(NAME, spec, subdim=False, uops_sha=shas)
    dops.OPS.append(op)
    dops.CUSTOM_DVE_SPECS[NAME] = spec
    dops._SUB_OPCODE_FOR_NAME[NAME] = row
    return op


def _plan(S):
    """Chunk/og plan shared by _build and assemble.

    Blocks split into chunks of <=512 columns.  Chunks pair into groups
    (one 128-row elementwise pass each); block 0's ramp chunks run single
    mode.  Consecutive pair-mode groups coalesce pairwise into supers
    sharing [128,1024] PSUM tiles, their og slots at stride 512.
    Returns (chunks, OGW); each chunk: g, lo, hi, mode, row, opos, group,
    super (super id or -1)."""
    chunks = []
    for g, gw in enumerate(S):
        gw = int(gw)
        single = (g == 0) or (g in SINGLE_BLOCKS)
        if g == 0:
            # two small singles to fill the pipeline, then a pair group
            h1 = 256 + (gw - 256 + 1) // 2
            for lo, hi in ((0, 128), (128, 256)):
                chunks.append(dict(g=g, lo=lo, hi=hi, mode="s"))
            for lo, hi in ((256, h1), (h1, gw)):
                chunks.append(dict(g=g, lo=lo, hi=hi, mode="p"))
            continue
        elif g == NBLK - 1:
            cut = gw - 128
            bounds = [0, (cut + 1) // 2, cut, cut + (gw - cut + 1) // 2, gw]
        elif gw > 2 * PW:
            bounds = [(gw * k + 2) // 4 for k in range(5)]
        else:
            bounds = [0, (gw + 1) // 2, gw]
        m = "s" if single else "p"
        for lo, hi in zip(bounds[:-1], bounds[1:]):
            chunks.append(dict(g=g, lo=lo, hi=hi, mode=m))
        del m

    assert len(chunks) % 2 == 0
    groups = []
    for i in range(0, len(chunks), 2):
        a, b = chunks[i], chunks[i + 1]
        assert a["hi"] - a["lo"] <= PW and b["hi"] - b["lo"] <= PW
        a["row"], b["row"] = 0, 64
        a["group"] = b["group"] = i // 2
        groups.append((a, b))

    # custom-DVE outputs must start at partition 0 on hardware, so single
    # chunks each get their own og span at row 0; pair groups (whose divide
    # writes all 128 partitions) share one span with row halves
    opos = 0
    for a, b in groups:
        if a["mode"] == "s":
            a["row"] = b["row"] = 0
            a["opos"] = opos
            opos += a["hi"] - a["lo"]
            b["opos"] = opos
            opos += b["hi"] - b["lo"]
        else:
            w = max(a["hi"] - a["lo"], b["hi"] - b["lo"])
            a["opos"] = b["opos"] = opos
            opos += w
    return chunks, opos


def _build(S):
    """Compile the per-core program for per-round column counts S[8]."""
    import concourse.bacc as bacc
    import concourse.bass as bass
    import concourse.mybir as mybir
    import concourse.tile as tile
    from concourse.dve_ops import RECIP_APPROX_FAST_CONSTS

    f16 = mybir.dt.float16
    f32 = mybir.dt.float32
    Cp = mybir.ActivationFunctionType.Copy
    fdiv = _fused_div_op()
    s0 = RECIP_APPROX_FAST_CONSTS["s0"]
    s1 = RECIP_APPROX_FAST_CONSTS["s1"]

    off = np.concatenate([[0], np.cumsum(S)]).astype(int)
    NT = int(off[-1])
    TOT = NBLK * WCOLS + NT
    wpos = [int(off[g]) + g * WCOLS for g in range(NBLK)]
    hpos = [wpos[g] + WCOLS for g in range(NBLK)]

    chunks, OGW = _plan(S)
    groups = [(chunks[i], chunks[i + 1]) for i in range(0, len(chunks), 2)]

    spans = [(0, WCOLS + 256),
             (WCOLS + 256, hpos[0] + int(S[0])),
             (wpos[1], hpos[1] + int(S[1])),
             (wpos[2], hpos[2] + int(S[2])),
             (wpos[3], hpos[3] + int(S[3])),
             (wpos[4], hpos[4] + int(S[4])),
             (wpos[5], hpos[5] + int(S[5])),
             (wpos[6], hpos[6] + int(S[6])),
             (wpos[7], TOT)]

    nc = bacc.Bacc("TRN2", target_bir_lowering=False, debug=False)
    hbuf = nc.dram_tensor("hbuf", [CROWS, TOT], f16, kind="ExternalInput")
    obuf = nc.dram_tensor("obuf", [128, OGW], f16, kind="ExternalOutput")

    with tile.TileContext(nc) as tc:
        with tc.tile_pool(name="hp", bufs=1) as hp, \
             tc.tile_pool(name="op", bufs=1) as op, \
             tc.tile_pool(name="cp", bufs=6) as cpool, \
             tc.tile_pool(name="pp", bufs=8, space=bass.MemorySpace.PSUM) as pp:
            hall = hp.tile([CROWS, TOT], f16, tag="hall", name="hall")
            og = op.tile([128, OGW], f16, tag="og", name="og")
            for (lo, hi) in spans:
                nc.sync.dma_start(out=hall[:, lo:hi], in_=hbuf.ap()[:, lo:hi])

            def h_ap(ck):
                base = hpos[ck["g"]] + ck["lo"]
                return hall[:, base:base + (ck["hi"] - ck["lo"])]

            def wslice(ck, which):
                base = wpos[ck["g"]] + (64 if which == "c" else 0)
                n = WCOLS if which == "full" else 64
                base = wpos[ck["g"]] if which == "full" else base
                return hall[:, base:base + n]

            gi = 0
            while gi < len(groups):
                a, b = groups[gi]
                if a["mode"] == "s":
                    # two single-mode chunks, each a 64-row pipeline
                    for ck in (a, b):
                        wk = ck["hi"] - ck["lo"]
                        ps = pp.tile([128, PW], f32, tag="ps", name="psS")
                        nc.tensor.matmul(ps[:, 0:wk],
                                         wslice(ck, "full"), h_ap(ck),
                                         start=True, stop=True)
                        cs = cpool.tile([64, PW], f16, tag="css", name="css")
                        nc.scalar.activation(out=cs[:, 0:wk],
                                             in_=ps[64:128, 0:wk],
                                             func=Cp, bias=0.0, scale=1.0)
                        rb = ck["row"]
                        nc.vector._custom_dve(
                            fdiv,
                            out=og[rb:rb + 64, ck["opos"]:ck["opos"] + wk],
                            in0=cs[:, 0:wk], in1=ps[0:64, 0:wk],
                            s0=s0, s1=s1, imm2=0.0)
                    gi += 1
                    continue
                # pair group: divisor matmuls first so the eviction starts
                # early; psAB/psC one PSUM bank each
                w = max(a["hi"] - a["lo"], b["hi"] - b["lo"])
                psAB = pp.tile([128, PW], f32, tag="ps", name="psAB")
                psC = pp.tile([128, PW], f32, tag="ps", name="psC")
                for ck, rb in ((a, 0), (b, 64)):
                    wk = ck["hi"] - ck["lo"]
                    nc.tensor.matmul(psC[rb:rb + 64, 0:wk],
                                     wslice(ck, "c"), h_ap(ck),
                                     start=True, stop=True)
                cs = cpool.tile([128, PW], f16, tag="cs", name="cs")
                nc.scalar.activation(out=cs[:, 0:w], in_=psC[:, 0:w],
                                     func=Cp, bias=0.0, scale=1.0)
                for ck, rb in ((a, 0), (b, 64)):
                    wk = ck["hi"] - ck["lo"]
                    nc.tensor.matmul(psAB[rb:rb + 64, 0:wk],
                                     wslice(ck, "ab"), h_ap(ck),
                                     start=True, stop=True)
                nc.vector._custom_dve(
                    fdiv, out=og[:, a["opos"]:a["opos"] + w],
                    in0=cs[:, 0:w], in1=psAB[:, 0:w],
                    s0=s0, s1=s1, imm2=0.0)
                gi += 1

            # stores: og column ranges at group boundaries; SP early (free
            # after loads), the last two ranges on SP and Pool concurrently
            gb = sorted({c["opos"] for c in chunks} | {OGW})
            n = len(gb) - 1
            marks = sorted(set([gb[0], gb[(n + 7) // 8], gb[(n + 3) // 4],
                                gb[(n + 1) // 2], gb[(3 * n) // 4], gb[n - 5],
                                gb[n - 4], gb[n - 3], gb[n - 2], gb[n - 1],
                                gb[n]]))
            nst = len(marks) - 1
            engs = [nc.sync] * nst
            for k in range(2, nst - 1, 2):
                engs[k] = nc.gpsimd
            if nst >= 1:
                engs[nst - 1] = nc.scalar
            for i2 in range(nst):
                lo, hi = marks[i2], marks[i2 + 1]
                if hi <= lo:
                    continue
                engs[i2].dma_start(out=obuf.ap()[:, lo:hi], in_=og[:, lo:hi])
    nc.compile()
    return nc


def get_nc():
    return _CACHE["nc"]


def _schedule(counts):
    """Rank keyframes by count; block b holds ranks [32b,32b+32), core b%8,
    round b//8.  S[j] = max count in round j (shared SPMD column budget)."""
    order = np.argsort(-counts, kind="stable")
    sc = np.zeros(NRANK, dtype=np.int64)
    sc[:N_KF] = counts[order]
    bmax = sc.reshape(N_CORES * NBLK, KF_PER_BLK).max(axis=1)
    S = bmax.reshape(NBLK, N_CORES).max(axis=1)
    return order, S


def prepare(tMP, tKF, idxKF, idxMP, meas_kf, meas_mp):
    """Host-side shard/pack.  Returns (in_maps, unpack_state)."""
    M = meas_kf.shape[0]
    ikf = np.searchsorted(np.asarray(idxKF), np.asarray(meas_kf))
    imp = np.searchsorted(np.asarray(idxMP), np.asarray(meas_mp))

    counts = np.bincount(ikf, minlength=N_KF).astype(np.int64)
    order, S = _schedule(counts)
    off = np.concatenate([[0], np.cumsum(S)]).astype(np.int64)
    NT = int(off[-1])

    rank_of = np.empty(N_KF, dtype=np.int64)
    rank_of[order] = np.arange(N_KF)

    key = rank_of[ikf]
    perm = np.argsort(key, kind="stable")
    skey = key[perm]
    gstart = np.zeros(NRANK, dtype=np.int64)
    gstart[1:N_KF + 1] = np.cumsum(counts[order])
    occ = np.arange(M, dtype=np.int64) - gstart[skey]

    blk = skey // KF_PER_BLK
    core_s = blk % N_CORES
    j_s = blk // N_CORES
    lane_s = skey % KF_PER_BLK
    col_s = off[j_s] + occ

    # H streams: rows 3*lane+{0,1,2} = point coords, row 96 = ones
    T = np.asarray(tKF, dtype=np.float64)
    pts16 = np.asarray(tMP, dtype=np.float16)[imp[perm]]
    H = np.zeros((N_CORES, CROWS, NT), dtype=np.float16)
    H[:, CROWS - 1, :] = np.float16(1.0)
    r0 = 3 * lane_s
    H[core_s, r0, col_s] = pts16[:, 0]
    H[core_s, r0 + 1, col_s] = pts16[:, 1]
    H[core_s, r0 + 2, col_s] = pts16[:, 2]

    # weights: folded intrinsics rows per keyframe
    A = FX * T[:, 0, :] + CX * T[:, 2, :]          # [N_KF, 4]
    B = FY * T[:, 1, :] + CY * T[:, 2, :]
    C = T[:, 2, :]
    W = np.zeros((N_CORES, CROWS, NBLK * WCOLS), dtype=np.float16)
    r = np.arange(NRANK)
    rb = r // KF_PER_BLK
    rcore = rb % N_CORES
    rj = rb // N_CORES
    rlane = r % KF_PER_BLK
    base = rj * WCOLS
    # default c-bias 1 on dummy lanes so padded/dummy columns never divide by 0
    W[rcore, CROWS - 1, base + 64 + 2 * rlane] = np.float16(1.0)
    W[rcore, CROWS - 1, base + 65 + 2 * rlane] = np.float16(1.0)
    rr = r[:N_KF]
    kf = order
    for t in range(3):
        W[rcore[rr], 3 * rlane[rr] + t, base[rr] + 2 * rlane[rr]] = A[kf, t]
        W[rcore[rr], 3 * rlane[rr] + t, base[rr] + 2 * rlane[rr] + 1] = B[kf, t]
        W[rcore[rr], 3 * rlane[rr] + t, base[rr] + 64 + 2 * rlane[rr]] = C[kf, t]
        W[rcore[rr], 3 * rlane[rr] + t, base[rr] + 65 + 2 * rlane[rr]] = C[kf, t]
    W[rcore[rr], CROWS - 1, base[rr] + 2 * rlane[rr]] = A[kf, 3]
    W[rcore[rr], CROWS - 1, base[rr] + 2 * rlane[rr] + 1] = B[kf, 3]
    W[rcore[rr], CROWS - 1, base[rr] + 64 + 2 * rlane[rr]] = C[kf, 3]
    W[rcore[rr], CROWS - 1, base[rr] + 65 + 2 * rlane[rr]] = C[kf, 3]

    # device buffer columns: [W_0 | H_0 | W_1 | H_1 | ... | W_7 | H_7]
    NTW = NBLK * WCOLS
    HB = np.empty((N_CORES, CROWS, NTW + NT), dtype=np.float16)
    pos = 0
    for g in range(NBLK):
        gw = int(S[g])
        HB[:, :, pos:pos + WCOLS] = W[:, :, g * WCOLS:(g + 1) * WCOLS]
        HB[:, :, pos + WCOLS:pos + WCOLS + gw] = H[:, :, off[g]:off[g + 1]]
        pos += WCOLS + gw

    # og coordinate map per global H column (for assemble)
    chunks, OGW = _plan(S)
    orow = np.empty(NT, dtype=np.int64)
    ocol = np.empty(NT, dtype=np.int64)
    for ck in chunks:
        glo = off[ck["g"]] + ck["lo"]
        ghi = off[ck["g"]] + ck["hi"]
        orow[glo:ghi] = ck["row"]
        ocol[glo:ghi] = ck["opos"] + np.arange(ck["hi"] - ck["lo"])

    in_maps = [{"hbuf": HB[c]} for c in range(N_CORES)]
    unpack = (perm, core_s, lane_s, col_s, M, orow, ocol,
              tuple(int(s) for s in S))
    return in_maps, unpack


def assemble(results, unpack):
    perm, core_s, lane_s, col_s, M, orow, ocol, _ = unpack
    O = np.stack([np.asarray(results[c]["obuf"]) for c in range(N_CORES)])
    row = orow[col_s] + 2 * lane_s
    col = ocol[col_s]
    out = np.empty((M, 2), dtype=np.float32)
    out[perm, 0] = O[core_s, row, col].astype(np.float32)
    out[perm, 1] = O[core_s, row + 1, col].astype(np.float32)
    return out


def kernel(tMP, tKF, idxKF, idxMP, meas_kf, meas_mp):
    import time

    from concourse.bass_utils import run_bass_kernel_spmd

    in_maps, unpack = prepare(tMP, tKF, idxKF, idxMP, meas_kf, meas_mp)
    S = unpack[-1]
    if _CACHE.get("S") != S:
        _CACHE["nc"] = _build(S)
        _CACHE["S"] = S
    nc = _CACHE["nc"]
    # transient NRT exec-unit errors occur when a previous process was still
    # draining the cores; a cooldown plus reset recovers them
    last = None
    for attempt, delay in enumerate((0.0, 2.0, 45.0)):
        if delay:
            import os
            os.environ.setdefault("NEURON_RT_RESET_CORES", "1")
            time.sleep(delay)
        try:
            res = run_bass_kernel_spmd(nc, in_maps, core_ids=list(range(N_CORES)))
            return assemble(res.results, unpack)
        except Exception as e:
            last = e
    raise last
